# revision 30
# baseline (speedup 1.0000x reference)
"""Multi-head attention on 8 Trainium2 NeuronCores.

Problem: x[4,2048,1024] -> attention(16 heads, d=64) -> out proj -> [4,2048,1024].

Sharding: core c handles (batch b = c//2, sequence half s = c%2). Each core
computes q for its 1024 query rows and k/v for the full 2048 rows of its
batch (k/v recomputed by both half-cores — cheaper than a collective), so
cores are fully independent and the host just concatenates outputs.

Per-core dataflow (bf16 matmul operands, fp32 PSUM accumulation):
  x^T fed pre-transposed from host   [c, rows] bf16
  weights fed pre-swizzled from host in exact SBUF layout (bf16, contiguous
  partition lines) so weight DMAs are a handful of large-packet transfers
  q^T = Wq_pair^T @ x^T              [128(2 heads), 1024]
  k^T = Wk_pair^T @ x^T              [128(2 heads), 2048]
  v   = x^T.T @ Wv (4-head waves)    fp8, [jb, 4 heads, 80] groups with a
                                     ones column at col 64 of each head
  S^T = k_h^T-slices.T @ q_h^T       [j-block, i]   (K=64, 2 heads row-tiled)
  expS = exp(S^T * 0.125)            ScalarE, PSUM->SBUF fp8, [128,1024]
  out^T_aug = v_aug^T @ expS^T       fp8 DoubleRow: both j-blocks of a pair
                                     in one matmul; h -> po[0:65] (Z at 64)
  out^T = out^T_aug * (1/Z)          Z rows collected at partitions 0:2 via
                                     SBUF->SBUF DMAs, one reciprocal, one
                                     K=2 PE broadcast, one multiply ->
                                     persistent [128, NI] bf16 outT tile
  y = outT.T @ Wo + bo               chunks 0..6 pumped into pair 7's loop,
                                     chunk 7 split around the last norms

Scheduling: projections for pair p+1 and output-projection chunks are
emitted in small generator "pumps" inside the ACT-paced attention loop of
pair p, keeping the PE array dense so the HAM clock gate stays at 2.4 GHz.
Pair 0 bootstraps with the minimal prefix (q it0, k jt0/jt1, v jb0..3) and
pumps at double rate. The softmax normalization is split: the DVE/DMA half
runs inline (freeing PSUM), the PE broadcast + multiply are deferred into
the next i-tile so the in-order PE queue never stalls on the DVE chain.
"""

import sys

if "/opt/trn_rl_repo" not in sys.path:
    sys.path.insert(0, "/opt/trn_rl_repo")

import numpy as np

B = 4
NSEQ = 2048
C = 1024          # query/model dim
H = 16
DH = 64
NI = 1024         # query rows per core
NJ = 2048         # key rows per core
NCC = C // 128    # 8 contraction chunks
NJB = NJ // 128   # 16 j blocks
SCALE = DH ** -0.5

_CACHE = {}


def _build_program():
    import concourse.bass as bass
    import concourse.mybir as mybir
    import concourse.tile as tile
    from concourse import bacc

    f32 = mybir.dt.float32
    f32r = mybir.dt.float32r
    bf16 = mybir.dt.bfloat16
    f8 = mybir.dt.float8e4
    DR = mybir.MatmulPerfMode.DoubleRow
    EXP = mybir.ActivationFunctionType.Exp
    MULT = mybir.AluOpType.mult
    ADD = mybir.AluOpType.add

    nc = bacc.Bacc("TRN2", target_bir_lowering=False, debug=False, num_devices=8)

    x_d = nc.dram_tensor("x", [C, NSEQ], bf16, kind="ExternalInput").ap()
    e2_d = nc.dram_tensor("e2c", [2, 128], f32, kind="ExternalInput").ap()
    wqh_d = nc.dram_tensor("wqh", [H // 2, 128, C], bf16, kind="ExternalInput").ap()
    wkh_d = nc.dram_tensor("wkh", [H // 2, 128, C], bf16, kind="ExternalInput").ap()
    wvh_d = nc.dram_tensor("wvh", [H // 4, 128, 2048], bf16, kind="ExternalInput").ap()
    woh_d = nc.dram_tensor("woh", [2, 128, 4096], bf16, kind="ExternalInput").ap()
    bias_d = nc.dram_tensor("biash", [128, C], f32, kind="ExternalInput").ap()
    y_d = nc.dram_tensor("y", [NI, C], f32, kind="ExternalOutput").ap()

    with tile.TileContext(nc) as tc:
        with tc.tile_pool(name="sb", bufs=1) as sbp, \
             tc.tile_pool(name="ps", bufs=1, space="PSUM") as psp:

            ones_f32 = sbp.tile([128, 128], f32, tag="misc3", bufs=1)
            nc.gpsimd.memset(ones_f32[:], 1.0)

            # --- phase 0: load x^T (host pre-transposed), 2 stages --------
            xT = []
            for cc in range(NCC):
                xT.append(sbp.tile([128, NSEQ], bf16, tag="xT", bufs=8, name=f"xT{cc}"))
            for lo, hi in ((0, 1024), (1024, 2048)):
                for cc in range(NCC):
                    eng = nc.sync if cc % 2 == 0 else nc.scalar
                    eng.dma_start(
                        out=xT[cc][:, lo:hi],
                        in_=x_d[cc * 128:(cc + 1) * 128, lo:hi])

            # persistent out^T tiles: pair p -> [128 (h0 rows 0:64, h1 rows
            # 64:128), NI] bf16; consumed by the output projection.
            outT = [sbp.tile([128, NI], bf16, tag="outT", bufs=8, name=f"oT{p}")
                    for p in range(H // 2)]

            vq_tiles = {}
            qkT_tiles = {}
            wvq_tiles = {}

            pending_normB = []

            def _emit_normA(po0, po1, p, iq):
                """DVE/DMA half of the softmax normalization — no PE
                instructions, so the in-order PE queue never stalls on it."""
                psa = sbp.tile([128, 512], f32, tag="posb", bufs=4,
                               name=f"psa_{p}_{iq}")
                nc.vector.tensor_copy(out=psa[0:65, :], in_=po0[0:65, :])
                sb1 = sbp.tile([128, 512], f32, tag="posb", bufs=4,
                               name=f"sb1_{p}_{iq}")
                nc.vector.tensor_copy(out=sb1[0:65, :], in_=po1[0:65, :])
                zz = sbp.tile([2, 512], f32, tag="zz", bufs=4,
                              name=f"zz_{p}_{iq}")
                # read psa row 64 (Z0) before the h1 shift overwrites it
                nc.sync.dma_start(out=zz[0:1, :], in_=psa[64:65, :])
                nc.gpsimd.dma_start(out=zz[1:2, :], in_=sb1[64:65, :])
                nc.sync.dma_start(out=psa[64:128, :], in_=sb1[0:64, :])
                rz = sbp.tile([2, 512], f32, tag="zz", bufs=4,
                              name=f"rz_{p}_{iq}")
                nc.vector.reciprocal_approx_fast(out=rz[0:2, :], in_=zz[0:2, :])
                rzr = sbp.tile([2, 512], f32r, tag="zz", bufs=4,
                               name=f"rzr_{p}_{iq}")
                nc.vector.tensor_copy(out=rzr[:], in_=rz[0:2, :])
                pending_normB.append((psa, rzr, p, iq))

            def _flush_normB():
                """PE broadcast + multiply, emitted a few j-blocks into the
                NEXT i-tile so the rzr chain is long finished when the PE
                reaches the pz matmul."""
                while pending_normB:
                    psa, rzr, p, iq = pending_normB.pop(0)
                    sl = slice(iq * 512, (iq + 1) * 512)
                    pz = psp.tile([128, 512], f32, tag="pst", bufs=2,
                                  name=f"pz_{p}_{iq}")
                    nc.tensor.matmul(pz[:], e2[:], rzr[:],
                                     start=True, stop=True)
                    zb = sbp.tile([128, 512], f32, tag="zb", bufs=8,
                                  name=f"zb_{p}_{iq}")
                    nc.vector.tensor_copy(out=zb[:], in_=pz[:])
                    nc.vector.tensor_tensor(out=outT[p][:, sl],
                                            in0=psa[:], in1=zb[:], op=MULT)

            def proj_gen(p):
                """Emit pair p's projections in small chunks (generator) so
                they can be interleaved into the previous pair's attention
                loop. Pair 0 emits a yield-free bootstrap prefix first.
                The v-wave for qwave qw is split between its two pairs'
                generators (8 j-blocks each) to balance pump work."""
                boot = (p == 0)
                wqp = sbp.tile([128, C], bf16, tag="wqk", bufs=4,
                               name=f"wqp{p}")
                nc.gpsimd.dma_start(out=wqp[:], in_=wqh_d[p])
                wkp = sbp.tile([128, C], bf16, tag="wqk", bufs=4,
                               name=f"wkp{p}")
                nc.gpsimd.dma_start(out=wkp[:], in_=wkh_d[p])
                # wave ownership: pair 0 creates+fully projects wave 0;
                # afterwards odd pair p creates wave (p+1)//2 and projects
                # its first half, even pair p>0 completes wave p//2.
                wv_new = 0 if p == 0 else ((p + 1) // 2 if p % 2 == 1 else None)
                if wv_new is not None and wv_new < H // 4:
                    wvq = sbp.tile([128, 2048], bf16, tag="wvq", bufs=2,
                                   name=f"wvq{wv_new}")
                    wvq_tiles[wv_new] = wvq
                    nc.gpsimd.dma_start(out=wvq[:], in_=wvh_d[wv_new])
                    # fp8 v with 80-byte head stride (keeps the DoubleRow
                    # jb-pair dim 16B-aligned); col 64 of each head group is
                    # the softmax-denominator ones column.
                    vq = sbp.tile([128, NJB * 320], f8, tag="vq", bufs=2,
                                  name=f"vq{wv_new}")
                    vq_tiles[wv_new] = vq
                    nc.vector.tensor_copy(
                        out=vq[:].rearrange("p (jb h e) -> p jb h e",
                                            jb=NJB, h=4)[:, :, :, 64:65],
                        in_=ones_f32[:, 0:64].rearrange(
                            "p (a b c) -> p a b c", a=NJB, b=4))
                qT = sbp.tile([128, NI], bf16, tag="qT", bufs=3,
                              name=f"qT{p}")
                kT = sbp.tile([128, NJ], bf16, tag="kT", bufs=3,
                              name=f"kT{p}")
                qkT_tiles[p] = (qT, kT)

                def q_it(it, dy):
                    pq = psp.tile([128, 512], f32, tag="pst", bufs=2,
                                  name=f"pq{p}_{it}")
                    for cc in range(NCC):
                        nc.tensor.matmul(
                            pq[:], wqp[:, cc * 128:(cc + 1) * 128],
                            xT[cc][:, it * 512:(it + 1) * 512],
                            start=(cc == 0), stop=(cc == NCC - 1))
                        if dy and cc in (1, 3, 5):
                            yield
                    nc.vector.tensor_copy(
                        out=qT[:, it * 512:(it + 1) * 512], in_=pq[:])
                    if dy:
                        yield

                def k_jt(jt, dy):
                    pk = psp.tile([128, 512], f32, tag="pst", bufs=2,
                                  name=f"pk{p}_{jt}")
                    for cc in range(NCC):
                        nc.tensor.matmul(
                            pk[:], wkp[:, cc * 128:(cc + 1) * 128],
                            xT[cc][:, jt * 512:(jt + 1) * 512],
                            start=(cc == 0), stop=(cc == NCC - 1))
                        if dy and cc in (1, 3, 5):
                            yield
                    nc.vector.tensor_copy(
                        out=kT[:, jt * 512:(jt + 1) * 512], in_=pk[:])
                    if dy:
                        yield

                def v_jb(vq_w, wvq_w, jb, dy):
                    pv = psp.tile([128, 256], f32, tag="pst", bufs=2,
                                  name=f"pv{p}_{jb}")
                    for cc in range(NCC):
                        nc.tensor.matmul(
                            pv[:], xT[cc][:, jb * 128:(jb + 1) * 128],
                            wvq_w[:, cc * 256:(cc + 1) * 256],
                            start=(cc == 0), stop=(cc == NCC - 1))
                    nc.vector.tensor_copy(
                        out=vq_w[:].rearrange(
                            "p (jb h e) -> p jb h e", jb=NJB, h=4)
                        [:, jb, :, 0:64],
                        in_=pv[:].rearrange("p (h e) -> p h e", h=4))
                    if dy:
                        yield

                if p == 0:
                    vjbs, wv_p = range(4, NJB), 0
                elif p % 2 == 1 and (p + 1) // 2 < H // 4:
                    vjbs, wv_p = range(0, NJB // 2), (p + 1) // 2
                elif p % 2 == 0:
                    vjbs, wv_p = range(NJB // 2, NJB), p // 2
                else:
                    vjbs, wv_p = range(0), None

                if boot:
                    # minimal prefix for the pair-0 attention loop: q it0,
                    # k for j 0..1023, v for j-blocks 0..3 — emitted without
                    # yields, consumed by a single pre-loop pump.
                    yield from q_it(0, False)
                    for jt in (0, 1):
                        yield from k_jt(jt, False)
                    for jb in range(4):
                        yield from v_jb(vq_tiles[0], wvq_tiles[0], jb, False)
                    yield
                    for jt in (2, 3):
                        yield from k_jt(jt, True)
                    vq_w, wvq_w = vq_tiles[0], wvq_tiles[0]
                    for jb in vjbs:
                        yield from v_jb(vq_w, wvq_w, jb, True)
                    yield from q_it(1, True)
                else:
                    for it in range(NI // 512):
                        yield from q_it(it, True)
                    for jt in range(NJ // 512):
                        yield from k_jt(jt, True)
                    if wv_p is not None:
                        vq_w, wvq_w = vq_tiles.get(wv_p), wvq_tiles.get(wv_p)
                        for jb in vjbs:
                            yield from v_jb(vq_w, wvq_w, jb, True)

            def av_lhsT(vq, jbp, hq):
                return vq[:].rearrange("p (jb s) -> p jb s", jb=NJB)[
                    :, 2 * jbp:2 * jbp + 2, hq * 80:hq * 80 + 65]

            gens = {}

            def pump(key):
                g = gens.get(key)
                if g is not None and next(g, "done") == "done":
                    del gens[key]

            def pump_any(p):
                for key in (p, p + 1, "C"):
                    if key in gens:
                        pump(key)
                        return

            def drain_all(p):
                while any(k in gens for k in (p, p + 1, "C")):
                    pump_any(p)

            wo_holder = {}
            y_acc = {}

            def phasec_gen():
                """Pass 1 of the output projection (chunks 0..6) into an
                SBUF accumulator, interleaved into pair 7's attention."""
                wo_lo = sbp.tile([128, 4096], bf16, tag="wo", bufs=2)
                wo_hi = sbp.tile([128, 4096], bf16, tag="wo", bufs=2)
                wo_holder["lo"], wo_holder["hi"] = wo_lo, wo_hi
                nc.gpsimd.dma_start(out=wo_lo[:], in_=woh_d[0])
                nc.gpsimd.dma_start(out=wo_hi[:], in_=woh_d[1])
                yield
                for ib2 in range(NI // 128):
                    for eh in range(C // 512):
                        pc = psp.tile([128, 512], f32, tag="pst", bufs=2,
                                      name=f"pc{ib2}_{eh}")
                        for cc in range(7):
                            wo_t = wo_lo if cc < 4 else wo_hi
                            co = cc % 4
                            nc.tensor.matmul(
                                pc[:],
                                outT[cc][:, ib2 * 128:(ib2 + 1) * 128],
                                wo_t[:, co * 1024 + eh * 512:
                                     co * 1024 + eh * 512 + 512],
                                start=(cc == 0), stop=(cc == 6))
                            if cc == 3:
                                yield
                        ya = sbp.tile([128, 512], f32, tag="yacc", bufs=16,
                                      name=f"ya{ib2}_{eh}")
                        nc.vector.tensor_tensor(
                            out=ya[:], in0=pc[:],
                            in1=bias[:, eh * 512:(eh + 1) * 512], op=ADD)
                        y_acc[(ib2, eh)] = ya
                        yield

            def pass2_half(ib2s):
                """Output projection chunk 7 + accumulated pass 1 for half
                the i-range; adds and stores alternate engines/queues."""
                for ib2 in ib2s:
                    for eh in range(C // 512):
                        py = psp.tile([128, 512], f32, tag="pst", bufs=2,
                                      name=f"py{ib2}_{eh}")
                        nc.tensor.matmul(
                            py[:],
                            outT[7][:, ib2 * 128:(ib2 + 1) * 128],
                            wo_holder["hi"][:, 3 * 1024 + eh * 512:
                                            3 * 1024 + eh * 512 + 512],
                            start=True, stop=True)
                        ys = sbp.tile([128, 512], f32, tag="zb", bufs=8,
                                      name=f"ys{ib2}_{eh}")
                        nc.vector.tensor_tensor(
                            out=ys[:], in0=py[:],
                            in1=y_acc[(ib2, eh)][:], op=ADD)
                        deng = nc.sync if (ib2 + eh) % 2 == 0 else nc.scalar
                        deng.dma_start(
                            out=y_d[ib2 * 128:(ib2 + 1) * 128,
                                    eh * 512:(eh + 1) * 512],
                            in_=ys[:])

            # --- bootstrap pair 0, then late constants --------------------
            gens[0] = proj_gen(0)
            pump(0)     # emits weight DMAs + the bootstrap prefix

            e2s = sbp.tile([2, 128], f32, tag="misc4", bufs=1)
            nc.gpsimd.dma_start(out=e2s[:], in_=e2_d)
            # E2[K=2, M=128]: row 0 selects head-0 partitions 0:64, row 1
            # selects head-1 partitions 64:128 — one matmul broadcasts both
            # heads' 1/Z across their 64 output partitions.
            e2 = sbp.tile([2, 128], f32r, tag="misc2", bufs=1)
            nc.vector.tensor_copy(out=e2[:], in_=e2s[:])
            bias = sbp.tile([128, C], f32, tag="bias", bufs=1)
            nc.gpsimd.dma_start(out=bias[:], in_=bias_d)

            for p in range(H // 2):          # head pair index
                qw = p // 2
                if p + 1 < H // 2:
                    gens[p + 1] = proj_gen(p + 1)
                else:
                    gens["C"] = phasec_gen()
                qT, kT = qkT_tiles[p]

                # --- attention: both heads in lockstep --------------------
                # Adjacent S matmuls for the two heads use disjoint PE row
                # groups (rows 0:64 / 64:128) and execute concurrently.
                vq = vq_tiles[qw]
                for iq in range(NI // 512):
                    po = [psp.tile([128, 512], f32, tag="pso", bufs=2,
                                   name=f"po{p}_{iq}_{h2}") for h2 in range(2)]

                    def emit_av(jbp, ex2):
                        ex4 = ex2[:].rearrange("p (jo h i) -> p jo h i",
                                               jo=2, h=2)
                        for h2 in range(2):
                            hq = (p % 2) * 2 + h2
                            # fp8 DoubleRow: both jb of the pair in one
                            # matmul (contraction 256 over 128 partitions)
                            nc.tensor.matmul(
                                po[h2][0:65, :],
                                av_lhsT(vq, jbp, hq),
                                ex4[:, :, h2, :],
                                start=(jbp == 0), stop=(jbp == NJB // 2 - 1),
                                perf_mode=DR)

                    ex2 = None
                    ex_prev = None
                    for jb in range(NJB):
                        jbp, jo = jb // 2, jb % 2
                        if jb == 4:
                            _flush_normB()
                        if jb == 8 and p == 7 and iq == 1:
                            pass2_half(range(0, 4))
                        ps = psp.tile([128, 1024], f32, tag="pss", bufs=2)
                        for h2 in range(2):
                            d0 = h2 * 64
                            nc.tensor.matmul(
                                ps[:, h2 * 512:(h2 + 1) * 512],
                                kT[d0:d0 + 64, jb * 128:(jb + 1) * 128],
                                qT[d0:d0 + 64, iq * 512:(iq + 1) * 512],
                                start=True, stop=True)
                        if jo == 0:
                            ex2 = sbp.tile([128, 2048], f8, tag="exps",
                                           bufs=4, name=f"ex{p}_{iq}_{jbp}")
                        # fp8 exp tile, layout [jo, h2, i]
                        nc.scalar.activation(ex2[:, jo * 1024:(jo + 1) * 1024],
                                             ps[:], EXP, scale=SCALE)
                        if jo == 1:
                            # software-pipeline: AV for the PREVIOUS jb pair
                            # so its exp is long finished and the in-order PE
                            # queue never stalls waiting on ScalarE
                            if ex_prev is not None:
                                emit_av(jbp - 1, ex_prev)
                            ex_prev = ex2
                        pump_any(p)
                        if p == 0:
                            pump_any(p)
                    emit_av(NJB // 2 - 1, ex_prev)
                    _emit_normA(po[0], po[1], p, iq)
                drain_all(p)

            _flush_normB()
            pass2_half(range(4, NI // 128))

    nc.compile()
    return nc


def _get_program():
    if "nc" not in _CACHE:
        _CACHE["nc"] = _build_program()
    return _CACHE["nc"]


def _swizzle_weights(Wq, Wk, Wv, Wo, bo):
    import ml_dtypes
    bf = ml_dtypes.bfloat16

    def pair_blocks(W, width):
        n = W.shape[1] // width
        out = np.empty((n, 128, NCC * width), dtype=bf)
        for p in range(n):
            blk = W[:, p * width:(p + 1) * width]          # [1024, width]
            out[p] = blk.reshape(NCC, 128, width).transpose(1, 0, 2) \
                        .reshape(128, NCC * width).astype(bf)
        return np.ascontiguousarray(out)

    wqh = pair_blocks(Wq, 128)                             # [8, 128, 1024]
    wkh = pair_blocks(Wk, 128)
    wvh = pair_blocks(Wv, 256)                             # [4, 128, 2048]
    woh = np.empty((2, 128, 4096), dtype=bf)
    for half in range(2):
        blk = Wo[half * 512:(half + 1) * 512, :]           # [512, 1024]
        woh[half] = blk.reshape(4, 128, C).transpose(1, 0, 2) \
                       .reshape(128, 4096).astype(bf)
    biash = np.ascontiguousarray(
        np.broadcast_to(bo.astype(np.float32), (128, C)))
    return wqh, wkh, wvh, np.ascontiguousarray(woh), biash


def _make_in_maps(x, Wq, Wk, Wv, Wo, bo):
    import ml_dtypes
    x = np.ascontiguousarray(np.asarray(x, dtype=np.float32))
    wqh, wkh, wvh, woh, biash = _swizzle_weights(
        np.asarray(Wq, dtype=np.float32), np.asarray(Wk, dtype=np.float32),
        np.asarray(Wv, dtype=np.float32), np.asarray(Wo, dtype=np.float32),
        np.asarray(bo, dtype=np.float32))
    e2c = np.zeros((2, 128), dtype=np.float32)
    e2c[0, 0:64] = 1.0
    e2c[1, 64:128] = 1.0
    in_maps = []
    for c in range(8):
        b, s = c // 2, c % 2
        # rows 0..1023 of the per-core x are that core's query rows
        xb = x[b]
        xc = np.concatenate(
            [xb[s * NI:(s + 1) * NI], xb[(1 - s) * NI:(2 - s) * NI]], axis=0)
        in_maps.append({"x": np.ascontiguousarray(xc.T.astype(ml_dtypes.bfloat16)),
                        "wqh": wqh, "wkh": wkh, "wvh": wvh, "woh": woh,
                        "biash": biash, "e2c": e2c})
    return in_maps


def _assemble(results):
    out = np.empty((B, NSEQ, C), dtype=np.float32)
    for c in range(8):
        b, s = c // 2, c % 2
        out[b, s * NI:(s + 1) * NI] = results[c]["y"]
    return out


def kernel(x, Wq, Wk, Wv, Wo, bo):
    from concourse.bass_utils import run_bass_kernel_spmd

    nc = _get_program()
    in_maps = _make_in_maps(x, Wq, Wk, Wv, Wo, bo)
    res = run_bass_kernel_spmd(nc, in_maps, list(range(8)))
    return _assemble(res.results)


def kernel_traced(x, Wq, Wk, Wv, Wo, bo, tmpdir=None, trace_cores=None):
    """Like kernel() but also neuron-profiles; returns (out, exec_time_ns)."""
    from concourse.bass_utils import run_bass_kernel_spmd

    nc = _get_program()
    in_maps = _make_in_maps(x, Wq, Wk, Wv, Wo, bo)
    res = run_bass_kernel_spmd(nc, in_maps, list(range(8)), trace=True,
                               tmpdir=tmpdir, trace_cores=trace_cores)
    return _assemble(res.results), res.exec_time_ns


# revision 33
# speedup vs baseline: 1.0398x; 1.0398x over previous
"""Multi-head attention on 8 Trainium2 NeuronCores.

Problem: x[4,2048,1024] -> attention(16 heads, d=64) -> out proj -> [4,2048,1024].

Sharding: core c handles (batch b = c//2, sequence half s = c%2). Each core
computes q for its 1024 query rows and k/v for the full 2048 rows of its
batch (k/v recomputed by both half-cores — cheaper than a collective), so
cores are fully independent and the host just concatenates outputs.

Per-core dataflow (bf16 matmul operands, fp32 PSUM accumulation):
  x^T fed pre-transposed from host   [c, rows] bf16
  q^T = Wq_pair^T @ x^T              [128(2 heads), 1024]
  k^T = Wk_pair^T @ x^T              [128(2 heads), 2048]
  v   = x^T.T @ Wv (4-head waves)    [j, 4*65] with a ones column per head
                                     (h even: [v|1], h odd: [1|v])
  S^T = k_h^T-slices.T @ q_h^T       [j-block, i]   (K=64, 2 heads row-tiled)
  expS = exp(S^T * 0.125)            ScalarE, PSUM->SBUF, [128,1024] batches
  out^T_aug = v_aug^T @ expS^T       h0 -> po[0:65] (Z at 64),
                                     h1 -> po[63:128] (Z at 63)
  out^T = out^T_aug * (1/Z)          Z broadcast across partitions via PE;
                                     result lands partition-aligned in a
                                     persistent [128, NI] bf16 outT tile
  y = outT.T @ Wo + bo               [i, 1024]  (7 chunks pumped into pair
                                     7's loop, chunk 7 at the tail)

Scheduling: projections for pair p+1 and output-projection chunks are
emitted in small generator "pumps" inside the ACT-paced attention loop of
pair p, keeping the PE array dense so the HAM clock gate stays at 2.4 GHz.
The softmax normalization first drains po into SBUF (freeing the PSUM bank
for the next iq's AV immediately), then runs recip -> PE-broadcast ->
multiply entirely partition-aligned; out^T stays in SBUF (no DRAM bounce).
"""

import sys

if "/opt/trn_rl_repo" not in sys.path:
    sys.path.insert(0, "/opt/trn_rl_repo")

import numpy as np

B = 4
NSEQ = 2048
C = 1024          # query/model dim
H = 16
DH = 64
NI = 1024         # query rows per core
NJ = 2048         # key rows per core
NCC = C // 128    # 8 contraction chunks
NJB = NJ // 128   # 16 j blocks
SCALE = DH ** -0.5

_CACHE = {}


def _build_program():
    import concourse.bass as bass
    import concourse.mybir as mybir
    import concourse.tile as tile
    from concourse import bacc

    f32 = mybir.dt.float32
    f32r = mybir.dt.float32r
    bf16 = mybir.dt.bfloat16
    f8 = mybir.dt.float8e4
    DR = mybir.MatmulPerfMode.DoubleRow
    EXP = mybir.ActivationFunctionType.Exp
    MULT = mybir.AluOpType.mult
    ADD = mybir.AluOpType.add

    nc = bacc.Bacc("TRN2", target_bir_lowering=False, debug=False, num_devices=8)

    x_d = nc.dram_tensor("x", [C, NSEQ], bf16, kind="ExternalInput").ap()
    e2_d = nc.dram_tensor("e2c", [2, 128], f32, kind="ExternalInput").ap()
    wq_d = nc.dram_tensor("Wq", [C, H * DH], f32, kind="ExternalInput").ap()
    wk_d = nc.dram_tensor("Wk", [C, H * DH], f32, kind="ExternalInput").ap()
    wv_d = nc.dram_tensor("Wv", [C, H * DH], f32, kind="ExternalInput").ap()
    wo_d = nc.dram_tensor("Wo", [H * DH, C], f32, kind="ExternalInput").ap()
    bo_d = nc.dram_tensor("bo", [C], f32, kind="ExternalInput").ap()
    y_d = nc.dram_tensor("y", [NI, C], f32, kind="ExternalOutput").ap()

    def r(ap):
        return ap.bitcast(f32r)

    with tile.TileContext(nc) as tc:
        with tc.tile_pool(name="sb", bufs=1) as sbp, \
             tc.tile_pool(name="ps", bufs=1, space="PSUM") as psp:

            # --- constants -------------------------------------------------
            ones_f32 = sbp.tile([128, 128], f32, tag="misc3", bufs=1)
            nc.gpsimd.memset(ones_f32[:], 1.0)
            # E2[K=2, M=128]: row 0 selects head-0 partitions 0:64, row 1
            # selects head-1 partitions 64:128 — one matmul broadcasts both
            # heads' 1/Z across their 64 output partitions. Host constant.
            e2s = sbp.tile([2, 128], f32, tag="misc4", bufs=1)
            nc.gpsimd.dma_start(out=e2s[:], in_=e2_d)
            e2 = sbp.tile([2, 128], f32r, tag="misc2", bufs=1)
            nc.vector.tensor_copy(out=e2[:], in_=e2s[:])

            bias = sbp.tile([128, C], f32, tag="bias", bufs=1)
            nc.gpsimd.dma_start(out=bias[:],
                                in_=bo_d[None, :].to_broadcast((128, C)))

            # --- phase 0: load x^T (host pre-transposed) ------------------
            xT = []
            for cc in range(NCC):
                xT.append(sbp.tile([128, NSEQ], bf16, tag="xT", bufs=8, name=f"xT{cc}"))
            for lo, hi in ((0, 512), (512, 1024), (1024, 2048)):
                for cc in range(NCC):
                    # split the startup loads across both HWDGE queues so
                    # the first projection can start sooner
                    eng = nc.sync if cc % 2 == 0 else nc.scalar
                    eng.dma_start(
                        out=xT[cc][:, lo:hi],
                        in_=x_d[cc * 128:(cc + 1) * 128, lo:hi])

            wq34 = wq_d.rearrange("(cc p) e -> p cc e", p=128)
            wk34 = wk_d.rearrange("(cc p) e -> p cc e", p=128)
            wv34 = wv_d.rearrange("(cc p) e -> p cc e", p=128)

            # persistent out^T tiles: pair p -> [128 (h0 rows 0:64, h1 rows
            # 64:128), NI] bf16; consumed by the output projection.
            outT = [sbp.tile([128, NI], bf16, tag="outT", bufs=8, name=f"oT{p}")
                    for p in range(H // 2)]

            vq_tiles = {}
            qkT_tiles = {}
            wvq_tiles = {}

            pending_normB = []

            def _emit_normA(po0, po1, p, iq):
                """DVE/DMA half of the softmax normalization — no PE
                instructions, so the in-order PE queue never stalls on it.

                Both heads' AV results sit at partitions [0:65] of their po
                tile (out 0:64, Z at 64). Two aligned PSUM->SBUF copies free
                the po banks immediately; SBUF->SBUF DMAs then assemble an
                aligned [128,512] tile (h1 shifted to rows 64:128) and both
                Z rows at partitions 0:2 for one reciprocal.
                """
                psa = sbp.tile([128, 512], f32, tag="posb", bufs=4,
                               name=f"psa_{p}_{iq}")
                nc.vector.tensor_copy(out=psa[0:65, :], in_=po0[0:65, :])
                sb1 = sbp.tile([128, 512], f32, tag="posb", bufs=4,
                               name=f"sb1_{p}_{iq}")
                nc.vector.tensor_copy(out=sb1[0:65, :], in_=po1[0:65, :])
                zz = sbp.tile([2, 512], f32, tag="zz", bufs=4,
                              name=f"zz_{p}_{iq}")
                # read psa row 64 (Z0) before the h1 shift overwrites it
                nc.sync.dma_start(out=zz[0:1, :], in_=psa[64:65, :])
                nc.gpsimd.dma_start(out=zz[1:2, :], in_=sb1[64:65, :])
                nc.sync.dma_start(out=psa[64:128, :], in_=sb1[0:64, :])
                rz = sbp.tile([2, 512], f32, tag="zz", bufs=4,
                              name=f"rz_{p}_{iq}")
                nc.vector.reciprocal_approx_fast(out=rz[0:2, :], in_=zz[0:2, :])
                rzr = sbp.tile([2, 512], f32r, tag="zz", bufs=4,
                               name=f"rzr_{p}_{iq}")
                nc.vector.tensor_copy(out=rzr[:], in_=rz[0:2, :])
                pending_normB.append((psa, rzr, p, iq))

            def _flush_normB():
                """PE broadcast + multiply, emitted a few j-blocks into the
                NEXT i-tile so the rzr chain is long finished when the PE
                reaches the pz matmul."""
                while pending_normB:
                    psa, rzr, p, iq = pending_normB.pop(0)
                    sl = slice(iq * 512, (iq + 1) * 512)
                    pz = psp.tile([128, 512], f32, tag="pst", bufs=2,
                                  name=f"pz_{p}_{iq}")
                    nc.tensor.matmul(pz[:], e2[:], rzr[:],
                                     start=True, stop=True)
                    zb = sbp.tile([128, 512], f32, tag="zb", bufs=8,
                                  name=f"zb_{p}_{iq}")
                    nc.vector.tensor_copy(out=zb[:], in_=pz[:])
                    nc.vector.tensor_tensor(out=outT[p][:, sl],
                                            in0=psa[:], in1=zb[:], op=MULT)

            def proj_gen(p):
                """Emit pair p's projections in small chunks (generator) so
                they can be interleaved into the previous pair's
                ACT-paced attention loop, keeping the PE array dense.
                The v-wave for qwave qw is split between its two pairs'
                generators (8 j-blocks each) to balance pump work."""
                wqp = sbp.tile([128, C], bf16, tag="wqk", bufs=4,
                               name=f"wqp{p}")
                nc.gpsimd.dma_start(
                    out=wqp[:].rearrange("p (cc e) -> p cc e", cc=8),
                    in_=wq34[:, :, p * 128:(p + 1) * 128])
                wkp = sbp.tile([128, C], bf16, tag="wqk", bufs=4,
                               name=f"wkp{p}")
                nc.gpsimd.dma_start(
                    out=wkp[:].rearrange("p (cc e) -> p cc e", cc=8),
                    in_=wk34[:, :, p * 128:(p + 1) * 128])
                # wave ownership: pair 0 creates+fully projects wave 0;
                # afterwards odd pair p creates wave (p+1)//2 and projects
                # its first half, even pair p>0 completes wave p//2.
                wv_new = 0 if p == 0 else ((p + 1) // 2 if p % 2 == 1 else None)
                if wv_new is not None and wv_new < H // 4:
                    wvq = sbp.tile([128, 8 * 256], bf16, tag="wvq", bufs=2,
                                   name=f"wvq{wv_new}")
                    wvq_tiles[wv_new] = wvq
                    nc.gpsimd.dma_start(
                        out=wvq[:].rearrange("p (cc e) -> p cc e", cc=8),
                        in_=wv34[:, :, wv_new * 256:(wv_new + 1) * 256])
                    # fp8 v with 80-byte head stride (keeps the DoubleRow
                    # jb-pair dim 16B-aligned); col 64 of each head group is
                    # the softmax-denominator ones column.
                    vq = sbp.tile([128, NJB * 320], f8, tag="vq", bufs=2,
                                  name=f"vq{wv_new}")
                    vq_tiles[wv_new] = vq
                    nc.vector.tensor_copy(
                        out=vq[:].rearrange("p (jb h e) -> p jb h e",
                                            jb=NJB, h=4)[:, :, :, 64:65],
                        in_=ones_f32[:, 0:64].rearrange(
                            "p (a b c) -> p a b c", a=NJB, b=4))
                qT = sbp.tile([128, NI], bf16, tag="qT", bufs=3,
                              name=f"qT{p}")
                kT = sbp.tile([128, NJ], bf16, tag="kT", bufs=3,
                              name=f"kT{p}")
                qkT_tiles[p] = (qT, kT)
                for it in range(NI // 512):
                    pq = psp.tile([128, 512], f32, tag="pst", bufs=2,
                                  name=f"pq{p}_{it}")
                    for cc in range(NCC):
                        nc.tensor.matmul(
                            pq[:], wqp[:, cc * 128:(cc + 1) * 128],
                            xT[cc][:, it * 512:(it + 1) * 512],
                            start=(cc == 0), stop=(cc == NCC - 1))
                        if cc in (1, 3, 5):
                            yield
                    nc.vector.tensor_copy(
                        out=qT[:, it * 512:(it + 1) * 512], in_=pq[:])
                    yield
                for jt in range(NJ // 512):
                    pk = psp.tile([128, 512], f32, tag="pst", bufs=2,
                                  name=f"pk{p}_{jt}")
                    for cc in range(NCC):
                        nc.tensor.matmul(
                            pk[:], wkp[:, cc * 128:(cc + 1) * 128],
                            xT[cc][:, jt * 512:(jt + 1) * 512],
                            start=(cc == 0), stop=(cc == NCC - 1))
                        if cc in (1, 3, 5):
                            yield
                    nc.vector.tensor_copy(
                        out=kT[:, jt * 512:(jt + 1) * 512], in_=pk[:])
                    yield
                if p == 0:
                    vjbs, wv_p = range(NJB), 0
                elif p % 2 == 1 and (p + 1) // 2 < H // 4:
                    vjbs, wv_p = range(0, NJB // 2), (p + 1) // 2
                elif p % 2 == 0:
                    vjbs, wv_p = range(NJB // 2, NJB), p // 2
                else:
                    vjbs, wv_p = range(0), None
                if wv_p is not None:
                    vq_w, wvq_w = vq_tiles.get(wv_p), wvq_tiles.get(wv_p)
                for jb in vjbs:
                    pv = psp.tile([128, 256], f32, tag="pst", bufs=2,
                                  name=f"pv{p}_{jb}")
                    for cc in range(NCC):
                        nc.tensor.matmul(
                            pv[:], xT[cc][:, jb * 128:(jb + 1) * 128],
                            wvq_w[:, cc * 256:(cc + 1) * 256],
                            start=(cc == 0), stop=(cc == NCC - 1))
                        if cc == 3:
                            yield
                    nc.vector.tensor_copy(
                        out=vq_w[:].rearrange(
                            "p (jb h e) -> p jb h e", jb=NJB, h=4)
                        [:, jb, :, 0:64],
                        in_=pv[:].rearrange("p (h e) -> p h e", h=4))
                    yield

            def av_lhsT(vq, jbp, hq):
                return vq[:].rearrange("p (jb s) -> p jb s", jb=NJB)[
                    :, 2 * jbp:2 * jbp + 2, hq * 80:hq * 80 + 65]

            gens = {}

            def pump(p):
                g = gens.get(p)
                if g is not None and next(g, "done") == "done":
                    del gens[p]

            def drain(p):
                while p in gens:
                    pump(p)

            wo34 = wo_d.rearrange("(cc p) e -> p cc e", p=128)
            wo_holder = {}
            y_acc = {}

            def phasec_gen():
                """Pass 1 of the output projection (chunks 0..6) into an
                SBUF accumulator, interleaved into pair 7's attention."""
                wo_lo = sbp.tile([128, 4096], bf16, tag="wo", bufs=2)
                wo_hi = sbp.tile([128, 4096], bf16, tag="wo", bufs=2)
                wo_holder["lo"], wo_holder["hi"] = wo_lo, wo_hi
                nc.gpsimd.dma_start(
                    out=wo_lo[:].rearrange("p (cc e) -> p cc e", cc=4),
                    in_=wo34[:, 0:4, :])
                nc.gpsimd.dma_start(
                    out=wo_hi[:].rearrange("p (cc e) -> p cc e", cc=4),
                    in_=wo34[:, 4:8, :])
                yield
                for ib2 in range(NI // 128):
                    for eh in range(C // 512):
                        pc = psp.tile([128, 512], f32, tag="pst", bufs=2,
                                      name=f"pc{ib2}_{eh}")
                        for cc in range(7):
                            wo_t = wo_lo if cc < 4 else wo_hi
                            co = cc % 4
                            nc.tensor.matmul(
                                pc[:],
                                outT[cc][:, ib2 * 128:(ib2 + 1) * 128],
                                wo_t[:, co * 1024 + eh * 512:
                                     co * 1024 + eh * 512 + 512],
                                start=(cc == 0), stop=(cc == 6))
                            if cc in (2, 5):
                                yield
                        ya = sbp.tile([128, 512], f32, tag="yacc", bufs=16,
                                      name=f"ya{ib2}_{eh}")
                        nc.vector.tensor_tensor(
                            out=ya[:], in0=pc[:],
                            in1=bias[:, eh * 512:(eh + 1) * 512], op=ADD)
                        y_acc[(ib2, eh)] = ya
                        yield

            gens[0] = proj_gen(0)
            drain(0)

            for p in range(H // 2):          # head pair index
                qw = p // 2
                if p + 1 < H // 2:
                    gens[p + 1] = proj_gen(p + 1)
                else:
                    gens["C"] = phasec_gen()
                qT, kT = qkT_tiles[p]

                # --- attention: both heads in lockstep --------------------
                # Adjacent S matmuls for the two heads use disjoint PE row
                # groups (rows 0:64 / 64:128) and execute concurrently.
                vq = vq_tiles[qw]
                for iq in range(NI // 512):
                    po = [psp.tile([128, 512], f32, tag="pso", bufs=2,
                                   name=f"po{p}_{iq}_{h2}") for h2 in range(2)]

                    def emit_av(jbp, ex2):
                        ex4 = ex2[:].rearrange("p (jo h i) -> p jo h i",
                                               jo=2, h=2)
                        for h2 in range(2):
                            hq = (p % 2) * 2 + h2
                            # fp8 DoubleRow: both jb of the pair in one
                            # matmul (contraction 256 over 128 partitions)
                            nc.tensor.matmul(
                                po[h2][0:65, :],
                                av_lhsT(vq, jbp, hq),
                                ex4[:, :, h2, :],
                                start=(jbp == 0), stop=(jbp == NJB // 2 - 1),
                                perf_mode=DR)

                    ex2 = None
                    ex_prev = None
                    for jb in range(NJB):
                        jbp, jo = jb // 2, jb % 2
                        if jb == 4:
                            _flush_normB()
                        ps = psp.tile([128, 1024], f32, tag="pss", bufs=2)
                        for h2 in range(2):
                            d0 = h2 * 64
                            nc.tensor.matmul(
                                ps[:, h2 * 512:(h2 + 1) * 512],
                                kT[d0:d0 + 64, jb * 128:(jb + 1) * 128],
                                qT[d0:d0 + 64, iq * 512:(iq + 1) * 512],
                                start=True, stop=True)
                        if jo == 0:
                            ex2 = sbp.tile([128, 2048], f8, tag="exps",
                                           bufs=4, name=f"ex{p}_{iq}_{jbp}")
                        # fp8 exp tile, layout [jo, h2, i]
                        nc.scalar.activation(ex2[:, jo * 1024:(jo + 1) * 1024],
                                             ps[:], EXP, scale=SCALE)
                        if jo == 1:
                            # software-pipeline: AV for the PREVIOUS jb pair
                            # so its exp is long finished and the in-order PE
                            # queue never stalls waiting on ScalarE
                            if ex_prev is not None:
                                emit_av(jbp - 1, ex_prev)
                            ex_prev = ex2
                        if (p + 1) in gens:
                            pump(p + 1)
                        else:
                            pump("C")
                    emit_av(NJB // 2 - 1, ex_prev)
                    _emit_normA(po[0], po[1], p, iq)
                drain(p + 1)

            drain("C")
            _flush_normB()

            # --- phase C pass 2: chunk 7 + accumulated pass 1 -------------
            for ib2 in range(NI // 128):
                for eh in range(C // 512):
                    py = psp.tile([128, 512], f32, tag="pst", bufs=2,
                                  name=f"py{ib2}_{eh}")
                    nc.tensor.matmul(
                        py[:],
                        outT[7][:, ib2 * 128:(ib2 + 1) * 128],
                        wo_holder["hi"][:, 3 * 1024 + eh * 512:
                                        3 * 1024 + eh * 512 + 512],
                        start=True, stop=True)
                    ys = sbp.tile([128, 512], f32, tag="zb", bufs=8,
                                  name=f"ys{ib2}_{eh}")
                    nc.vector.tensor_tensor(
                        out=ys[:], in0=py[:],
                        in1=y_acc[(ib2, eh)][:], op=ADD)
                    nc.sync.dma_start(
                        out=y_d[ib2 * 128:(ib2 + 1) * 128,
                                eh * 512:(eh + 1) * 512],
                        in_=ys[:])

    nc.compile()
    return nc


def _get_program():
    if "nc" not in _CACHE:
        _CACHE["nc"] = _build_program()
    return _CACHE["nc"]


def _make_in_maps(x, Wq, Wk, Wv, Wo, bo):
    import ml_dtypes
    x = np.ascontiguousarray(np.asarray(x, dtype=np.float32))
    Wq = np.ascontiguousarray(np.asarray(Wq, dtype=np.float32))
    Wk = np.ascontiguousarray(np.asarray(Wk, dtype=np.float32))
    Wv = np.ascontiguousarray(np.asarray(Wv, dtype=np.float32))
    Wo = np.ascontiguousarray(np.asarray(Wo, dtype=np.float32))
    bo = np.ascontiguousarray(np.asarray(bo, dtype=np.float32))
    e2c = np.zeros((2, 128), dtype=np.float32)
    e2c[0, 0:64] = 1.0
    e2c[1, 64:128] = 1.0
    in_maps = []
    for c in range(8):
        b, s = c // 2, c % 2
        # rows 0..1023 of the per-core x are that core's query rows
        xb = x[b]
        xc = np.concatenate(
            [xb[s * NI:(s + 1) * NI], xb[(1 - s) * NI:(2 - s) * NI]], axis=0)
        in_maps.append({"x": np.ascontiguousarray(xc.T.astype(ml_dtypes.bfloat16)),
                        "Wq": Wq, "Wk": Wk, "Wv": Wv, "Wo": Wo, "bo": bo,
                        "e2c": e2c})
    return in_maps


def _assemble(results):
    out = np.empty((B, NSEQ, C), dtype=np.float32)
    for c in range(8):
        b, s = c // 2, c % 2
        out[b, s * NI:(s + 1) * NI] = results[c]["y"]
    return out


def kernel(x, Wq, Wk, Wv, Wo, bo):
    from concourse.bass_utils import run_bass_kernel_spmd

    nc = _get_program()
    in_maps = _make_in_maps(x, Wq, Wk, Wv, Wo, bo)
    res = run_bass_kernel_spmd(nc, in_maps, list(range(8)))
    return _assemble(res.results)


def kernel_traced(x, Wq, Wk, Wv, Wo, bo, tmpdir=None, trace_cores=None):
    """Like kernel() but also neuron-profiles; returns (out, exec_time_ns)."""
    from concourse.bass_utils import run_bass_kernel_spmd

    nc = _get_program()
    in_maps = _make_in_maps(x, Wq, Wk, Wv, Wo, bo)
    res = run_bass_kernel_spmd(nc, in_maps, list(range(8)), trace=True,
                               tmpdir=tmpdir, trace_cores=trace_cores)
    return _assemble(res.results), res.exec_time_ns
